# revision 6
# baseline (speedup 1.0000x reference)
"""Diagonal SSM kernel (Vandermonde contraction) on 8 Trainium2 NeuronCores.

Math: K[d,h,l] = 2*Re( sum_n sc[d,h,n] * w[h,n]^l ),  l in [0, 2048)
  where w = exp(a*dt), sc = c * (exp(a*dt)-1)/a.

Sharding: d_model (H=1024) split contiguously, 128 channels per core.

Strategy (per core): split l = J*c + j with J=64, c < 32. Host precomputes
(float64) two fp16 tables so the device does NO transcendentals:
  JT[64*h2 + 2n + t, 64p + j] = {Re,Im}(w[2p+h2, n]^j)        (basis)
  WT[64*h2 + 2n + t, 64p + 32d + c] = {2Re,-2Im}(sc * w^(64c)) (weights)
Device: per pair p two matmuls [k=64, m=64=(d,c), n=64=j] on diagonal PE
quadrants write PSUM partitions (h2, d, c) x columns j; 8 pairs fill one
PSUM bank [128, 512]f32; one copy per bank evacuates to a dense
[128, 4096]f16 staging tile; 4 wide DMAs ship it out. All DMAs move
>=2KB per partition line at full DMA-engine rate, and total HBM traffic
is 3 MB/core (vs 9.6 MB for the naive row-padded layout).
"""
from contextlib import ExitStack

import numpy as np

import concourse.bass as bass
import concourse.bacc as bacc
import concourse.tile as tile
from concourse import mybir
from concourse.bass_utils import run_bass_kernel_spmd

N_CORES = 8
H = 1024          # d_model
N = 32            # d_state//2
D = 2             # directions
L = 2048          # sequence length
J = 64            # j-block
CBLK = L // J     # 32 coarse blocks
HC = H // N_CORES     # 128 channels per core
NPAIR = HC // 2       # 64 pairs per core
NCHUNK = 4            # jt load chunks
PPC = NPAIR // NCHUNK     # 16 pairs per chunk
NBANK = 8                 # PSUM banks
PPB = NPAIR // NBANK      # 8 pairs per bank

JT_DT = "float16"         # basis table dtype on device
WT_DT = "float16"         # weight table dtype on device

_nc_cache = {}


def _build_nc(repeat: int = 1, jt_dt: str = None, wt_dt: str = None):
    """Build the Bass program. `repeat` re-runs the whole compute for timing."""
    jt_dt = jt_dt or JT_DT
    wt_dt = wt_dt or WT_DT
    key = (repeat, jt_dt, wt_dt)
    if key in _nc_cache:
        return _nc_cache[key]
    nc = bacc.Bacc("TRN2", target_bir_lowering=False, debug=False,
                   num_devices=N_CORES)
    f16 = mybir.dt.float16
    f32 = mybir.dt.float32
    djt = getattr(mybir.dt, jt_dt)
    dwt = getattr(mybir.dt, wt_dt)

    jt_d = nc.dram_tensor("jt", [128, NPAIR * J], djt, kind="ExternalInput")
    wt_d = nc.dram_tensor("wt", [128, NPAIR * 64], dwt, kind="ExternalInput")
    out_d = nc.dram_tensor("out", [128, NPAIR * J], f16, kind="ExternalOutput")

    with tile.TileContext(nc) as tc:
        with ExitStack() as ctx:
            wt_pool = ctx.enter_context(tc.tile_pool(name="wt", bufs=4))
            jt_pool = ctx.enter_context(tc.tile_pool(name="jt", bufs=2 * NCHUNK))
            ob_pool = ctx.enter_context(tc.tile_pool(name="ob", bufs=2))
            ps_pool = ctx.enter_context(
                tc.tile_pool(name="ps", bufs=NBANK, space="PSUM"))

            for _ in range(repeat):
                # all input DMAs on the sync queue so iteration i+1's loads
                # queue directly behind iteration i's (outputs go on scalar)
                wts_t, jts = [], []
                for half in range(2):
                    t = wt_pool.tile([128, NPAIR * 32], dwt, tag=f"wt{half}")
                    wts_t.append(t)
                for ch in range(NCHUNK):
                    t = jt_pool.tile([128, PPC * J], djt, tag="jt")
                    jts.append(t)
                nc.sync.dma_start(wts_t[0][:], wt_d.ap()[:, :NPAIR * 32])
                nc.sync.dma_start(jts[0][:], jt_d.ap()[:, :PPC * J])
                nc.sync.dma_start(jts[1][:], jt_d.ap()[:, PPC * J:2 * PPC * J])
                nc.sync.dma_start(wts_t[1][:], wt_d.ap()[:, NPAIR * 32:])
                nc.sync.dma_start(jts[2][:], jt_d.ap()[:, 2 * PPC * J:3 * PPC * J])
                nc.sync.dma_start(jts[3][:], jt_d.ap()[:, 3 * PPC * J:])
                ob = ob_pool.tile([128, NPAIR * J], f16, tag="ob")

                ps = None
                for p in range(NPAIR):
                    b, col = p // PPB, (p % PPB) * J
                    if p % PPB == 0:
                        ps = ps_pool.tile([128, PPB * J], f32)
                    jt = jts[p // PPC]
                    wt = wts_t[p // (NPAIR // 2)]
                    wc = (p % (NPAIR // 2)) * 64
                    pc = (p % PPC) * J
                    for h2 in (0, 1):
                        nc.tensor.matmul(
                            ps[64 * h2:64 * h2 + 64, col:col + J],
                            wt[64 * h2:64 * h2 + 64, wc:wc + 64],
                            jt[64 * h2:64 * h2 + 64, pc:pc + J],
                            start=True, stop=True,
                            tile_position=(64 * h2, 64 * h2),
                            skip_group_check=True,
                        )
                    if p % PPB == PPB - 1:
                        sl = slice(b * PPB * J, (b + 1) * PPB * J)
                        nc.vector.tensor_copy(ob[:, sl], ps[:])
                        if b % 2 == 1:
                            osl = slice((b - 1) * PPB * J, (b + 1) * PPB * J)
                            nc.scalar.dma_start(out_d.ap()[:, osl], ob[:, osl])
    nc.compile()
    _nc_cache[key] = nc
    return nc


def _host_tables(log_dt, log_a_real, a_imag, coeffs, jt_dt: str = None,
                 wt_dt: str = None):
    """Per-core JT/WT tables in float64 -> device dtypes."""
    np_jt = mybir.dt.np(getattr(mybir.dt, jt_dt or JT_DT))
    np_wt = mybir.dt.np(getattr(mybir.dt, wt_dt or WT_DT))
    dt = np.exp(log_dt.astype(np.float64))                       # [H]
    a = -np.exp(log_a_real.astype(np.float64)) + 1j * a_imag.astype(np.float64)
    da = a * dt[:, None]                                         # [H,N] c128
    c = coeffs[..., 0].astype(np.float64) + 1j * coeffs[..., 1].astype(np.float64)
    sc = c * (np.expm1(da) / a)[None]                            # [D,H,N]

    j = np.arange(J, dtype=np.float64)
    WjR = np.exp(da.real[:, :, None] * j) * np.cos(da.imag[:, :, None] * j)
    WjI = np.exp(da.real[:, :, None] * j) * np.sin(da.imag[:, :, None] * j)

    cs = np.arange(CBLK, dtype=np.float64)
    wJc = np.exp(da[:, :, None] * (J * cs))                      # [H,N,C]
    sig = sc[:, :, :, None] * wJc[None]                          # [D,H,N,C]

    jts, wts = [], []
    for core in range(N_CORES):
        h0 = core * HC
        # JT rows (h2, n, t), cols (p, j)
        jt = np.empty((2, N, 2, NPAIR, J), np.float64)
        R = WjR[h0:h0 + HC].reshape(NPAIR, 2, N, J).transpose(1, 2, 0, 3)
        I = WjI[h0:h0 + HC].reshape(NPAIR, 2, N, J).transpose(1, 2, 0, 3)
        jt[:, :, 0] = R
        jt[:, :, 1] = I
        jts.append(jt.reshape(128, NPAIR * J).astype(np_jt))

        # WT rows (h2, n, t), cols (p, d, c)
        s = sig[:, h0:h0 + HC].reshape(D, NPAIR, 2, N, CBLK)     # [d,p,h2,n,c]
        wt = np.empty((2, N, 2, NPAIR, D, CBLK), np.float64)
        wt[:, :, 0] = 2.0 * s.real.transpose(2, 3, 1, 0, 4)      # [h2,n,p,d,c]
        wt[:, :, 1] = -2.0 * s.imag.transpose(2, 3, 1, 0, 4)
        wts.append(wt.reshape(128, NPAIR * 64).astype(np_wt))
    return jts, wts


def _gather(results):
    """Assemble [D, H, L] f32 from per-core outs [128, NPAIR*J] f16."""
    outs = []
    for c in range(N_CORES):
        o = np.asarray(results[c]["out"]).astype(np.float32)
        # rows (h2, d, c), cols (p, j) -> [d, (p, h2), (c, j)]
        arr = o.reshape(2, D, CBLK, NPAIR, J).transpose(1, 3, 0, 2, 4)
        outs.append(arr.reshape(D, HC, L))
    return np.concatenate(outs, axis=1)


def kernel(log_dt, log_a_real, a_imag, coeffs, sequence_length, _repeat=1,
           _run=None):
    assert int(sequence_length) == L
    log_dt = np.asarray(log_dt)
    log_a_real = np.asarray(log_a_real)
    a_imag = np.asarray(a_imag)
    coeffs = np.asarray(coeffs)
    jts, wts = _host_tables(log_dt, log_a_real, a_imag, coeffs)
    nc = _build_nc(_repeat)
    in_maps = [{"jt": jts[c], "wt": wts[c]} for c in range(N_CORES)]
    run = _run or (lambda n, m: run_bass_kernel_spmd(
        n, m, core_ids=list(range(N_CORES)), trace=False).results)
    results = run(nc, in_maps)
    return _gather(results)


def emulate(log_dt, log_a_real, a_imag, coeffs, sequence_length):
    """Numpy emulation of the device program (quantized tables, f32 accum)."""
    assert int(sequence_length) == L
    jts, wts = _host_tables(log_dt, log_a_real, a_imag, coeffs)
    results = []
    for core in range(N_CORES):
        jt = jts[core].astype(np.float32)
        wt = wts[core].astype(np.float32)
        out = np.zeros((128, NPAIR * J), np.float32)
        for p in range(NPAIR):
            for h2 in (0, 1):
                blk = wt[64 * h2:64 * h2 + 64, 64 * p:64 * p + 64].T \
                    @ jt[64 * h2:64 * h2 + 64, p * J:(p + 1) * J]
                out[64 * h2:64 * h2 + 64, p * J:(p + 1) * J] = blk
        results.append({"out": out.astype(np.float16)})
    return _gather(results)


# revision 12
# speedup vs baseline: 1.8307x; 1.8307x over previous
"""Diagonal SSM kernel (Vandermonde contraction) on 8 Trainium2 NeuronCores.

Math: K[d,h,l] = 2*Re( sum_n sc[d,h,n] * w[h,n]^l ),  l in [0, 2048)
  where w = exp(a*dt), sc = c * (exp(a*dt)-1)/a.

Sharding: d_model (H=1024) split contiguously, 128 channels per core.

Strategy (per core): split l = J*c + j with J=64, c < 32. Host precomputes
(float64) two fp16 tables so the device does NO transcendentals:
  JT[64*h2 + 2n + t, 64p + j] = {Re,Im}(w[2p+h2, n]^j)        (basis)
  WT[64*h2 + 2n + t, 64p + 32d + c] = {2Re,-2Im}(sc * w^(64c)) (weights)
Device: per pair p two matmuls [k=64, m=64=(d,c), n=64=j] on diagonal PE
quadrants write PSUM partitions (h2, d, c) x columns j; 8 pairs fill one
PSUM bank [128, 512]f32; one copy per bank evacuates to a dense
[128, 4096]f16 staging tile; 4 wide DMAs ship it out. All DMAs move
>=2KB per partition line at full DMA-engine rate, and total HBM traffic
is 3 MB/core (vs 9.6 MB for the naive row-padded layout).
"""
from contextlib import ExitStack

import numpy as np

import concourse.bass as bass
import concourse.bacc as bacc
import concourse.tile as tile
from concourse import mybir
from concourse.bass_utils import run_bass_kernel_spmd

N_CORES = 8
H = 1024          # d_model
N = 32            # d_state//2
D = 2             # directions
L = 2048          # sequence length
J = 64            # j-block
CBLK = L // J     # 32 coarse blocks
HC = H // N_CORES     # 128 channels per core
NPAIR = HC // 2       # 64 pairs per core
NCHUNK = 4            # jt load chunks
PPC = NPAIR // NCHUNK     # 16 pairs per chunk
NBANK = 8                 # PSUM banks
PPB = NPAIR // NBANK      # 8 pairs per bank

JT_DT = "float16"         # basis table dtype on device
WT_DT = "float16"         # weight table dtype on device
RUNROLL = 8               # compute bodies per hardware-loop iteration

_nc_cache = {}


def _emit_body(nc, wt_pool, jt_pool, ob_pool, ps_pool, jt_d, wt_d, out_d,
               djt, dwt):
    f16 = mybir.dt.float16
    f32 = mybir.dt.float32
    # all input DMAs on the sync queue so body i+1's loads queue directly
    # behind body i's (outputs go on the scalar queue)
    wts_t, jts = [], []
    for half in range(2):
        wts_t.append(wt_pool.tile([128, NPAIR * 32], dwt, tag=f"wt{half}", name=f"wt{half}"))
    for ch in range(NCHUNK):
        jts.append(jt_pool.tile([128, PPC * J], djt, tag="jt", name="jt"))
    nc.sync.dma_start(wts_t[0][:], wt_d.ap()[:, :NPAIR * 32])
    nc.sync.dma_start(jts[0][:], jt_d.ap()[:, :PPC * J])
    nc.sync.dma_start(jts[1][:], jt_d.ap()[:, PPC * J:2 * PPC * J])
    nc.sync.dma_start(wts_t[1][:], wt_d.ap()[:, NPAIR * 32:])
    nc.sync.dma_start(jts[2][:], jt_d.ap()[:, 2 * PPC * J:3 * PPC * J])
    nc.sync.dma_start(jts[3][:], jt_d.ap()[:, 3 * PPC * J:])
    ob = ob_pool.tile([128, NPAIR * J], f16, tag="ob", name="ob")

    ps = None
    for p in range(NPAIR):
        b, col = p // PPB, (p % PPB) * J
        if p % PPB == 0:
            ps = ps_pool.tile([128, PPB * J], f32, name="ps")
        jt = jts[p // PPC]
        wt = wts_t[p // (NPAIR // 2)]
        wc = (p % (NPAIR // 2)) * 64
        pc = (p % PPC) * J
        for h2 in (0, 1):
            nc.tensor.matmul(
                ps[64 * h2:64 * h2 + 64, col:col + J],
                wt[64 * h2:64 * h2 + 64, wc:wc + 64],
                jt[64 * h2:64 * h2 + 64, pc:pc + J],
                start=True, stop=True,
                tile_position=(64 * h2, 64 * h2),
                skip_group_check=True,
            )
        if p % PPB == PPB - 1:
            sl = slice(b * PPB * J, (b + 1) * PPB * J)
            # alternate evac engines: DVE serializing all 8 banks would bind
            if b % 2 == 0:
                nc.vector.tensor_copy(ob[:, sl], ps[:])
            else:
                nc.scalar.copy(ob[:, sl], ps[:])
            if b % 2 == 1:
                osl = slice((b - 1) * PPB * J, (b + 1) * PPB * J)
                nc.scalar.dma_start(out_d.ap()[:, osl], ob[:, osl])


def _build_nc(repeat: int = 1, jt_dt: str = None, wt_dt: str = None):
    """Build the Bass program. `repeat` re-runs the whole compute for timing
    (python-unrolled; hardware For_i loops crash the exec unit under this
    runtime)."""
    jt_dt = jt_dt or JT_DT
    wt_dt = wt_dt or WT_DT
    key = (repeat, jt_dt, wt_dt)
    if key in _nc_cache:
        return _nc_cache[key]
    nc = bacc.Bacc("TRN2", target_bir_lowering=False, debug=False,
                   num_devices=N_CORES)
    djt = getattr(mybir.dt, jt_dt)
    dwt = getattr(mybir.dt, wt_dt)

    jt_d = nc.dram_tensor("jt", [128, NPAIR * J], djt, kind="ExternalInput")
    wt_d = nc.dram_tensor("wt", [128, NPAIR * 64], dwt, kind="ExternalInput")
    out_d = nc.dram_tensor("out", [128, NPAIR * J], mybir.dt.float16,
                           kind="ExternalOutput")

    with tile.TileContext(nc) as tc:
        with ExitStack() as ctx:
            wt_pool = ctx.enter_context(tc.tile_pool(name="wt", bufs=4))
            jt_pool = ctx.enter_context(tc.tile_pool(name="jt", bufs=2 * NCHUNK))
            ob_pool = ctx.enter_context(tc.tile_pool(name="ob", bufs=2))
            ps_pool = ctx.enter_context(
                tc.tile_pool(name="ps", bufs=NBANK, space="PSUM"))

            for _ in range(repeat):
                _emit_body(nc, wt_pool, jt_pool, ob_pool, ps_pool,
                           jt_d, wt_d, out_d, djt, dwt)
    nc.compile()
    _nc_cache[key] = nc
    return nc


def _host_tables(log_dt, log_a_real, a_imag, coeffs, jt_dt: str = None,
                 wt_dt: str = None):
    """Per-core JT/WT tables in float64 -> device dtypes."""
    np_jt = mybir.dt.np(getattr(mybir.dt, jt_dt or JT_DT))
    np_wt = mybir.dt.np(getattr(mybir.dt, wt_dt or WT_DT))
    dt = np.exp(log_dt.astype(np.float64))                       # [H]
    a = -np.exp(log_a_real.astype(np.float64)) + 1j * a_imag.astype(np.float64)
    da = a * dt[:, None]                                         # [H,N] c128
    c = coeffs[..., 0].astype(np.float64) + 1j * coeffs[..., 1].astype(np.float64)
    sc = c * (np.expm1(da) / a)[None]                            # [D,H,N]

    j = np.arange(J, dtype=np.float64)
    WjR = np.exp(da.real[:, :, None] * j) * np.cos(da.imag[:, :, None] * j)
    WjI = np.exp(da.real[:, :, None] * j) * np.sin(da.imag[:, :, None] * j)

    cs = np.arange(CBLK, dtype=np.float64)
    wJc = np.exp(da[:, :, None] * (J * cs))                      # [H,N,C]
    sig = sc[:, :, :, None] * wJc[None]                          # [D,H,N,C]

    jts, wts = [], []
    for core in range(N_CORES):
        h0 = core * HC
        # JT rows (h2, n, t), cols (p, j)
        jt = np.empty((2, N, 2, NPAIR, J), np.float64)
        R = WjR[h0:h0 + HC].reshape(NPAIR, 2, N, J).transpose(1, 2, 0, 3)
        I = WjI[h0:h0 + HC].reshape(NPAIR, 2, N, J).transpose(1, 2, 0, 3)
        jt[:, :, 0] = R
        jt[:, :, 1] = I
        jts.append(jt.reshape(128, NPAIR * J).astype(np_jt))

        # WT rows (h2, n, t), cols (p, d, c)
        s = sig[:, h0:h0 + HC].reshape(D, NPAIR, 2, N, CBLK)     # [d,p,h2,n,c]
        wt = np.empty((2, N, 2, NPAIR, D, CBLK), np.float64)
        wt[:, :, 0] = 2.0 * s.real.transpose(2, 3, 1, 0, 4)      # [h2,n,p,d,c]
        wt[:, :, 1] = -2.0 * s.imag.transpose(2, 3, 1, 0, 4)
        wts.append(wt.reshape(128, NPAIR * 64).astype(np_wt))
    return jts, wts


def _gather(results):
    """Assemble [D, H, L] f32 from per-core outs [128, NPAIR*J] f16."""
    outs = []
    for c in range(N_CORES):
        o = np.asarray(results[c]["out"]).astype(np.float32)
        # rows (h2, d, c), cols (p, j) -> [d, (p, h2), (c, j)]
        arr = o.reshape(2, D, CBLK, NPAIR, J).transpose(1, 3, 0, 2, 4)
        outs.append(arr.reshape(D, HC, L))
    return np.concatenate(outs, axis=1)


def kernel(log_dt, log_a_real, a_imag, coeffs, sequence_length, _repeat=1,
           _run=None):
    assert int(sequence_length) == L
    log_dt = np.asarray(log_dt)
    log_a_real = np.asarray(log_a_real)
    a_imag = np.asarray(a_imag)
    coeffs = np.asarray(coeffs)
    jts, wts = _host_tables(log_dt, log_a_real, a_imag, coeffs)
    nc = _build_nc(_repeat)
    in_maps = [{"jt": jts[c], "wt": wts[c]} for c in range(N_CORES)]
    run = _run or (lambda n, m: run_bass_kernel_spmd(
        n, m, core_ids=list(range(N_CORES)), trace=False).results)
    results = run(nc, in_maps)
    return _gather(results)


def emulate(log_dt, log_a_real, a_imag, coeffs, sequence_length):
    """Numpy emulation of the device program (quantized tables, f32 accum)."""
    assert int(sequence_length) == L
    jts, wts = _host_tables(log_dt, log_a_real, a_imag, coeffs)
    results = []
    for core in range(N_CORES):
        jt = jts[core].astype(np.float32)
        wt = wts[core].astype(np.float32)
        out = np.zeros((128, NPAIR * J), np.float32)
        for p in range(NPAIR):
            for h2 in (0, 1):
                blk = wt[64 * h2:64 * h2 + 64, 64 * p:64 * p + 64].T \
                    @ jt[64 * h2:64 * h2 + 64, p * J:(p + 1) * J]
                out[64 * h2:64 * h2 + 64, p * J:(p + 1) * J] = blk
        results.append({"out": out.astype(np.float16)})
    return _gather(results)
